# revision 8
# baseline (speedup 1.0000x reference)
"""Dynamic per-pixel 3x3 filtering on 8 Trainium2 NeuronCores.

out[b,c,y,x] = sum_{ki,kj} img[b,c,y+ki-1,x+kj-1] * kernels[b,c,ki*3+kj,y,x]
(zero padding outside the image).

Sharding: pure data parallel, one batch sample per core (B=8, 8 cores).

Per-core layout: each channel's [512, 512] image plane is viewed as
[128 partitions, 4 blocks, 512 cols] (row r = block*128 + partition).
Row-shifted variants (y-1 / y+1) are loaded from HBM with row-offset
access patterns. Every bulk DMA uses exactly 128 partitions: the HWDGE
only stripes a transfer across the 16 SDMA engines when the SBUF side
spans all 128 partitions; anything else lands entirely on SDMA engine 0
(measured), which serializes the kernel. The two edge blocks (needing
the zero rows y=-1 / y=512) therefore go through a small padded scratch
in internal DRAM, written/read as full-128-partition transfers. Column
shifts are free-dim AP offsets.

All 17 elementwise passes (9 mult + 8 accumulate) run on the Vector
engine: concurrent GPSIMD tensor_tensor work contends with DVE for the
shared SBUF port (measured 2.5x DVE slowdown), so a tap split across
engines loses. DMA issue is split across both HWDGE sequencers (SP for
image/shift traffic, ACT for kernel-tile loads and stores) because a
single sequencer serializes on per-DMA descriptor generation.
"""

from contextlib import ExitStack

import numpy as np

import concourse.bacc as bacc
import concourse.mybir as mybir
import concourse.tile as tile
from concourse.bass_utils import run_bass_kernel_spmd

C, H, W = 3, 512, 512
KK = 9
NCORES = 8
P = 128
NB = H // P          # 4 row blocks per channel
FW = NB * W          # 2048 free-dim width of a channel mega-tile
F32 = mybir.dt.float32

# Taps: t = ki*3 + kj; row shift = ki-1 (top/mid/bot), col shift = kj-1.
# mid taps first (no shift-DMA dependency); first tap must be dx=0 (full write).
TAP_ORDER = [4, 3, 5, 1, 0, 2, 7, 6, 8]


def _r3(ap):
    """[128, FW] -> [128, NB, W] block view of a channel mega-tile."""
    return ap.rearrange("p (b x) -> p b x", x=W)


def _emit(nc, tc, ctx):
    img = nc.dram_tensor("img", (C, H, W), F32, kind="ExternalInput").ap()
    ker = nc.dram_tensor("kernels", (C, KK, H, W), F32, kind="ExternalInput").ap()
    out = nc.dram_tensor("out", (C, H, W), F32, kind="ExternalOutput").ap()

    v_pool = ctx.enter_context(tc.tile_pool(name="v", bufs=2))
    z_pool = ctx.enter_context(tc.tile_pool(name="z", bufs=1))
    k_pool = ctx.enter_context(tc.tile_pool(name="k", bufs=14))
    acc_pool = ctx.enter_context(tc.tile_pool(name="acc", bufs=2))
    tmp_pool = ctx.enter_context(tc.tile_pool(name="tmp", bufs=1))
    dram_pool = ctx.enter_context(tc.tile_pool(name="dpad", bufs=2, space="DRAM"))

    zrow = z_pool.tile([P, W], F32, tag="zrow")
    nc.gpsimd.memset(zrow[:, :], 0.0)

    def emit_loads(c):
        # img rows for this channel: mid[p, b*W + x] = img[c, b*128 + p, x]
        mid = v_pool.tile([P, FW], F32, tag="mid")
        nc.sync.dma_start(
            _r3(mid[:, :]), img[c].rearrange("(b p) x -> p b x", p=P)
        )
        # top[p, b, x] = img row (b*128 + p - 1): blocks 1..3 (rows 127..510)
        # are affine-valid straight from img; block 0 (rows -1..126) goes
        # through the padded DRAM scratch (needs the zero row above the image).
        top = v_pool.tile([P, FW], F32, tag="top")
        nc.sync.dma_start(
            top[:, W:FW].rearrange("p (b x) -> p b x", x=W),
            img[c, P - 1 : H - 1].rearrange("(b p) x -> p b x", p=P),
        )
        # bot[p, b, x] = img row (b*128 + p + 1): blocks 0..2 from img,
        # block 3 (rows 385..512) via the scratch (zero row below the image).
        bot = v_pool.tile([P, FW], F32, tag="bot")
        nc.sync.dma_start(
            bot[:, 0 : FW - W].rearrange("p (b x) -> p b x", x=W),
            img[c, 1 : 1 + H - P].rearrange("(b p) x -> p b x", p=P),
        )
        # Padded scratch strip: pad[1 + r] = img[c, r]; rows 0 / 513 zeroed.
        # Only rows 0..128 and 385..513 are ever read back.
        pad = dram_pool.tile([H + 2, W], F32, tag="pad")
        nc.sync.dma_start(pad[0:1, :], zrow[0:1, :])
        nc.sync.dma_start(pad[H + 1 : H + 2, :], zrow[0:1, :])
        nc.sync.dma_start(pad[1 : P + 1, :], mid[:, 0:W])
        nc.sync.dma_start(pad[H - P + 1 : H + 1, :], mid[:, FW - W : FW])
        nc.sync.dma_start(top[:, 0:W], pad[0:P, :])
        nc.sync.dma_start(bot[:, FW - W : FW], pad[H - P + 2 : H + 2, :])

        kts = {}
        for t in TAP_ORDER:
            kt = k_pool.tile([P, FW], F32, tag="kt")
            nc.scalar.dma_start(
                _r3(kt[:, :]), ker[c, t].rearrange("(b p) x -> p b x", p=P)
            )
            kts[t] = kt
        return mid, top, bot, kts

    def emit_compute_store(c, tiles):
        mid, top, bot, kts = tiles
        acc = acc_pool.tile([P, FW], F32, tag="acc")
        tmp = tmp_pool.tile([P, FW], F32, tag="tmp")
        vs = [top, mid, bot]
        eng = nc.vector
        first = True
        for t in TAP_ORDER:
            ki, kj = divmod(t, 3)
            v, dx = vs[ki], kj - 1
            if dx == 0:
                if first:
                    eng.tensor_mul(acc[:, :], v[:, :], kts[t][:, :])
                else:
                    eng.tensor_mul(tmp[:, :], v[:, :], kts[t][:, :])
                    eng.tensor_add(acc[:, :], acc[:, :], tmp[:, :])
            else:
                a3, v3, k3 = _r3(acc[:, :]), _r3(v[:, :]), _r3(kts[t][:, :])
                tsl = _r3(tmp[:, :])[:, :, 0 : W - 1]
                if dx < 0:
                    asl, vsl, ksl = a3[:, :, 1:W], v3[:, :, 0 : W - 1], k3[:, :, 1:W]
                else:
                    asl, vsl, ksl = a3[:, :, 0 : W - 1], v3[:, :, 1:W], k3[:, :, 0 : W - 1]
                eng.tensor_mul(tsl, vsl, ksl)
                eng.tensor_add(asl, asl, tsl)
            first = False
        nc.sync.dma_start(
            out[c].rearrange("(b p) x -> p b x", p=P), _r3(acc[:, :])
        )

    # Software-pipelined emission: channel c+1's loads enter the DMA rings
    # BEFORE channel c's store, so the store's wait-for-compute never blocks
    # the next channel's loads (HWDGE rings are FIFO).
    tiles = emit_loads(0)
    for c in range(C):
        nxt = emit_loads(c + 1) if c + 1 < C else None
        emit_compute_store(c, tiles)
        tiles = nxt


_NC_CACHE = []


def _build():
    nc = bacc.Bacc(
        "TRN2",
        target_bir_lowering=False,
        debug=False,
        enable_asserts=True,
        num_devices=1,
    )
    with tile.TileContext(nc) as tc:
        with ExitStack() as ctx:
            _emit(nc, tc, ctx)
    nc.compile()
    return nc


def kernel(img, kernels):
    """img: [8, 3, 512, 512] f32; kernels: [8, 3, 9, 512, 512] f32.
    Returns [8, 3, 512, 512] f32."""
    if not _NC_CACHE:
        _NC_CACHE.append(_build())
    nc = _NC_CACHE[0]
    img = np.asarray(img, dtype=np.float32)
    kernels = np.asarray(kernels, dtype=np.float32)
    in_maps = [
        {
            "img": np.ascontiguousarray(img[b]),
            "kernels": np.ascontiguousarray(kernels[b]),
        }
        for b in range(NCORES)
    ]
    res = run_bass_kernel_spmd(nc, in_maps, core_ids=list(range(NCORES)))
    return np.stack([res.results[b]["out"] for b in range(NCORES)], axis=0)
